# revision 1
# baseline (speedup 1.0000x reference)
"""ChessGNN (2-layer GCN + global max pool + FC + log_softmax) on 8 Trainium2 cores.

Strategy (edge-parallel, dst-range sharded):
  - Core k owns dst nodes [12500k, 12500(k+1)). Edges are routed to the core
    owning their dst. Within a core, edges are ordered by (src-range, dst-bucket)
    where a bucket is 128 consecutive dst nodes.
  - Per layer: every core computes hs = (h_prev @ W) * dinv for its node slice,
    writes it to a padded [12544, 64] buffer, AllGather -> [100352, 64] table.
  - Messages are fetched with the GPSIMD dma_gather (1024 idxs / instruction,
    int16 idxs => 4 src ranges of 32768 rows).
  - Segment-sum is matmul-based: per 128-edge chunk, a one-hot [128e, 128slot]
    matrix (DVE is_equal vs iota) is multiplied with the 128x32 message block on
    the PE, accumulating per-bucket in PSUM. No scatter primitives (their CCE
    add loses updates on duplicate indices).
  - deg is computed with the same one-hot trick (rhs = ones). dinv = 1/sqrt(deg+1).
  - Head: local max over slice -> PE transpose -> AllReduce(max) -> FC -> log_softmax.
"""
import numpy as np

import concourse.bass as bass
import concourse.bacc as bacc
import concourse.mybir as mybir
import concourse.tile as tile
from concourse.bass_utils import run_bass_kernel_spmd
from concourse.masks import make_identity

N = 100000
NCORES = 8
S = N // NCORES            # 12500 nodes per core
NB = 98                    # buckets of 128 dst nodes (98*128 = 12544)
SP = NB * 128              # padded slice rows
NPAD = NCORES * SP         # padded global rows = 100352
RNG = 32768                # int16 gather range
NRANGES = (NPAD + RNG - 1) // RNG  # 4
PADDLOC = 999.0

LAST_RESULTS = None
RUN_WALL_NS = None


def _prep_core(src_g, dl, ranges_cnt_max=None):
    """Order core edges by (src-range, bucket); return per-(g,b) counts or padded arrays."""
    rg = src_g >> 15
    b = dl >> 7
    order = np.lexsort((b, rg))
    return src_g[order], dl[order], rg[order], b[order]


def kernel(x, edge_index, W1, b1, W2, b2, fcW, fcb):
    global LAST_RESULTS
    x = np.asarray(x, np.float32)
    ei = np.asarray(edge_index)
    src = ei[0].astype(np.int64)
    dst = ei[1].astype(np.int64)

    # ---- host sharding / marshaling ----
    owner = dst // S
    src_gid = (src // S) * SP + (src % S)      # padded global row id of src
    per = []
    for k in range(NCORES):
        m = owner == k
        sg, dl, rg, bb = _prep_core(src_gid[m], (dst[m] - k * S).astype(np.int64))
        per.append((sg, dl, rg, bb))

    # per (g, b) chunk counts = max over cores, rounded to 128
    cnt = np.zeros((NCORES, NRANGES, NB), np.int64)
    for k in range(NCORES):
        sg, dl, rg, bb = per[k]
        np.add.at(cnt[k], (rg, bb), 1)
    chunks = (cnt.max(axis=0) + 127) // 128            # [NRANGES, NB]
    # pad each range's chunk count to a multiple of 8 (1024-idx gather instrs)
    cg = chunks.sum(axis=1)
    extra = (-cg) % 8
    chunks[:, NB - 1] += extra
    cg = chunks.sum(axis=1)                             # chunks per range
    C = int(cg.sum())                                   # total chunks
    NI = [int(c) // 8 for c in cg]                      # gather instrs per range
    NI_tot = sum(NI)

    # build per-core padded idx/dloc arrays in the global chunk grid
    g16_all, dlf_all = [], []
    for k in range(NCORES):
        sg, dl, rg, bb = per[k]
        gidx = np.zeros(C * 128, np.int16)
        dloc = np.full(C * 128, PADDLOC, np.float32)
        pos = 0
        ptr = 0
        for g in range(NRANGES):
            for b in range(NB):
                n = int(cnt[k, g, b])
                cap = int(chunks[g, b]) * 128
                sl = slice(ptr, ptr + n)
                gidx[pos:pos + n] = (sg[sl] - g * RNG).astype(np.int16)
                dloc[pos:pos + n] = (dl[sl] - b * 128).astype(np.float32)
                ptr += n
                pos += cap
        # gather idx slabs: instruction m covers idxs [1024m,1024(m+1)),
        # idx i -> [i%16, i//16] of a [16,64] slab, replicated to 128 partitions.
        slabs = gidx.reshape(NI_tot, 64, 16).transpose(0, 2, 1)      # [NI,16,64]
        g16 = np.tile(slabs.reshape(1, NI_tot * 16, 64)
                      .reshape(NI_tot, 16, 64), (1, 8, 1)).transpose(1, 0, 2) \
                .reshape(128, NI_tot * 64)
        # dloc layout: chunk j, lane p -> [p, j]
        dlf = dloc.reshape(C, 128).T.copy()
        g16_all.append(np.ascontiguousarray(g16))
        dlf_all.append(dlf)

    xT_all = []
    for k in range(NCORES):
        xs = np.zeros((SP, 8), np.float32)
        xs[:S] = x[k * S:(k + 1) * S]
        xT_all.append(np.ascontiguousarray(xs.T))

    iota = np.tile(np.arange(128, dtype=np.float32), (128, 1))
    b1t = np.tile(np.asarray(b1, np.float32)[None, :], (128, 1))
    b2t = np.tile(np.asarray(b2, np.float32)[None, :], (128, 1))
    fcb2 = np.asarray(fcb, np.float32)[None, :]

    # ---- build the SPMD program ----
    nc = bacc.Bacc("TRN2", target_bir_lowering=False, debug=False, num_devices=NCORES)
    dt = mybir.dt
    xT_t = nc.dram_tensor("xT", [8, SP], dt.float32, kind="ExternalInput")
    g16_t = nc.dram_tensor("g16", [128, NI_tot * 64], dt.int16, kind="ExternalInput")
    dlf_t = nc.dram_tensor("dlf", [128, C], dt.float32, kind="ExternalInput")
    iota_t = nc.dram_tensor("iota", [128, 128], dt.float32, kind="ExternalInput")
    W1_t = nc.dram_tensor("W1", [8, 32], dt.float32, kind="ExternalInput")
    W2_t = nc.dram_tensor("W2", [32, 32], dt.float32, kind="ExternalInput")
    b1_t = nc.dram_tensor("b1t", [128, 32], dt.float32, kind="ExternalInput")
    b2_t = nc.dram_tensor("b2t", [128, 32], dt.float32, kind="ExternalInput")
    fcW_t = nc.dram_tensor("fcW", [32, 5], dt.float32, kind="ExternalInput")
    fcb_t = nc.dram_tensor("fcb", [1, 5], dt.float32, kind="ExternalInput")
    out_t = nc.dram_tensor("out", [1, 5], dt.float32, kind="ExternalOutput")

    AF = mybir.ActivationFunctionType
    ALU = mybir.AluOpType
    AX = mybir.AxisListType

    with tile.TileContext(nc) as tc:
        with (
            tc.tile_pool(name="per", bufs=1) as per_p,
            tc.tile_pool(name="gt", bufs=4) as gt_p,
            tc.tile_pool(name="oh", bufs=4) as oh_p,
            tc.tile_pool(name="ps", bufs=2, space="PSUM") as ps_p,
            tc.tile_pool(name="psd", bufs=1, space="PSUM") as psd_p,
            tc.tile_pool(name="dram", bufs=1, space="DRAM") as dram_p,
        ):
            # persistent tiles
            xT = per_p.tile([8, SP], dt.float32)
            G16 = per_p.tile([128, NI_tot * 64], dt.int16)
            DLF = per_p.tile([128, C], dt.float32)
            IO = per_p.tile([128, 128], dt.float32)
            W1s = per_p.tile([8, 32], dt.float32)
            W2s = per_p.tile([32, 32], dt.float32)
            B1 = per_p.tile([128, 32], dt.float32)
            B2 = per_p.tile([128, 32], dt.float32)
            FCW = per_p.tile([32, 5], dt.float32)
            FCB = per_p.tile([1, 5], dt.float32)
            ONES = per_p.tile([128, 1], dt.float32)
            IDN = per_p.tile([128, 128], dt.float32)
            P = per_p.tile([128, NB, 32], dt.float32)    # h_prev @ W (slice)
            H = per_p.tile([128, NB, 32], dt.float32)    # layer output
            HS = per_p.tile([128, NB, 32], dt.float32)   # P * dinv
            ACC = per_p.tile([128, NB, 32], dt.float32)  # aggregated messages
            DEG = per_p.tile([128, NB], dt.float32)
            DINV = per_p.tile([128, NB], dt.float32)

            for t_, s_ in ((xT, xT_t), (G16, g16_t), (DLF, dlf_t), (IO, iota_t),
                           (W1s, W1_t), (W2s, W2_t), (B1, b1_t), (B2, b2_t),
                           (FCW, fcW_t), (FCB, fcb_t)):
                nc.sync.dma_start(t_[:], s_[:, :])
            nc.vector.memset(ONES[:], 1.0)
            make_identity(nc, IDN[:])

            agin1 = dram_p.tile([SP, 64], dt.float32)
            agout1 = dram_p.tile([NPAD, 64], dt.float32)
            agin2 = dram_p.tile([SP, 64], dt.float32)
            agout2 = dram_p.tile([NPAD, 64], dt.float32)
            arin = dram_p.tile([32, 1], dt.float32)
            arout = dram_p.tile([32, 1], dt.float32)

            # ---- P1 = x @ W1 (per 128-node tile) ----
            for t in range(NB):
                psm = ps_p.tile([128, 32], dt.float32, tag="pmm")
                nc.tensor.matmul(psm[:], lhsT=xT[:, t * 128:(t + 1) * 128],
                                 rhs=W1s[:], start=True, stop=True)
                nc.scalar.copy(P[:, t, :], psm[:])

            # ---- deg via one-hot matmuls (rhs = ones) ----
            nc.vector.memset(DEG[:], 0.0)
            jg = 0
            psd = None
            for g in range(NRANGES):
                flat = []
                for b in range(NB):
                    nch = int(chunks[g, b])
                    for c in range(nch):
                        flat.append((b, c == 0, c == nch - 1))
                for m in range(len(flat) // 8):
                    oh = oh_p.tile([128, 8, 128], dt.float32, tag="oha")
                    j0 = jg + m * 8
                    nc.vector.tensor_tensor(
                        out=oh[:],
                        in0=DLF[:, j0:j0 + 8].rearrange("p (c o) -> p c o", o=1)
                            .to_broadcast([128, 8, 128]),
                        in1=IO[:].rearrange("p (o s) -> p o s", o=1)
                            .to_broadcast([128, 8, 128]),
                        op=ALU.is_equal)
                    for s in range(8):
                        b, first, last = flat[m * 8 + s]
                        if first:
                            psd = psd_p.tile([128, 1], dt.float32, tag="pdeg")
                        nc.tensor.matmul(psd[:], lhsT=oh[:, s, :], rhs=ONES[:],
                                         start=first, stop=last)
                        if last:
                            nc.vector.tensor_add(DEG[:, b:b + 1], DEG[:, b:b + 1], psd[:])
                jg += len(flat)
            # dinv = 1/sqrt(deg + 1)
            SQ = per_p.tile([128, NB], dt.float32)
            nc.scalar.activation(SQ[:], DEG[:], AF.Sqrt, bias=1.0)
            nc.vector.reciprocal(DINV[:], SQ[:])

            dinv_b = DINV[:].rearrange("p (b o) -> p b o", o=1).to_broadcast([128, NB, 32])

            def aggregate(agout, acc):
                """gather + one-hot matmul segment sum over all chunks."""
                nc.vector.memset(acc[:], 0.0)
                jg2 = 0
                mi = 0
                for g in range(NRANGES):
                    r0 = g * RNG
                    r1 = min((g + 1) * RNG, NPAD)
                    src_ap = agout[r0:r1, :]
                    # per-range schedule: (bucket, count) pairs
                    sched = [(b, int(chunks[g, b])) for b in range(NB) if chunks[g, b] > 0]
                    flat = []
                    for b, nch in sched:
                        for c in range(nch):
                            flat.append((b, c == 0, c == nch - 1))
                    ntiles = len(flat) // 8
                    for m in range(ntiles):
                        gt = gt_p.tile([128, 8, 64], dt.float32, tag="gt")
                        nc.gpsimd.dma_gather(gt[:], src_ap, G16[:, mi * 64:(mi + 1) * 64],
                                             1024, 1024, 64)
                        oh = oh_p.tile([128, 8, 128], dt.float32, tag="oha")
                        j0 = jg2 + m * 8
                        nc.vector.tensor_tensor(
                            out=oh[:],
                            in0=DLF[:, j0:j0 + 8].rearrange("p (c o) -> p c o", o=1)
                                .to_broadcast([128, 8, 128]),
                            in1=IO[:].rearrange("p (o s) -> p o s", o=1)
                                .to_broadcast([128, 8, 128]),
                            op=ALU.is_equal)
                        for s in range(8):
                            b, first, last = flat[m * 8 + s]
                            if first:
                                psm = ps_p.tile([128, 32], dt.float32, tag="pagg")
                            nc.tensor.matmul(psm[:], lhsT=oh[:, s, :], rhs=gt[:, s, 0:32],
                                             start=first, stop=last)
                            if last:
                                nc.vector.tensor_add(acc[:, b, :], acc[:, b, :], psm[:])
                        mi += 1
                    jg2 += len(flat)

            def combine(acc, Pt, Bt, h):
                """h = relu(dinv*(acc + dinv*P) + b)"""
                T1 = per_p.tile([128, NB, 32], dt.float32, tag="t1")
                nc.vector.tensor_mul(T1[:], Pt[:], dinv_b)
                nc.vector.tensor_add(T1[:], T1[:], acc[:])
                nc.vector.tensor_mul(T1[:], T1[:], dinv_b)
                nc.vector.tensor_add(
                    T1[:], T1[:],
                    Bt[:].rearrange("p (o f) -> p o f", o=1).to_broadcast([128, NB, 32]))
                nc.scalar.activation(h[:], T1[:], AF.Relu)

            # ---- layer 1 ----
            nc.vector.tensor_mul(HS[:], P[:], dinv_b)
            nc.sync.dma_start(
                agin1[:, :].rearrange("(a p) b -> p a b", p=128)[:, :, 0:32], HS[:])
            nc.gpsimd.collective_compute(
                "AllGather", ALU.bypass, replica_groups=[list(range(NCORES))],
                ins=[agin1.opt()], outs=[agout1.opt()])
            aggregate(agout1, ACC)
            combine(ACC, P, B1, H)

            # ---- P2 = h1 @ W2 via per-tile transpose ----
            for t in range(NB):
                pst = psd_p.tile([32, 128], dt.float32, tag="ptr")
                nc.tensor.transpose(out=pst[:], in_=H[:, t, :], identity=IDN[:])
                h1t = gt_p.tile([32, 128], dt.float32, tag="h1t")
                nc.scalar.copy(h1t[:], pst[:])
                psm = ps_p.tile([128, 32], dt.float32, tag="pmm")
                nc.tensor.matmul(psm[:], lhsT=h1t[:], rhs=W2s[:], start=True, stop=True)
                nc.scalar.copy(P[:, t, :], psm[:])

            # ---- layer 2 ----
            nc.vector.tensor_mul(HS[:], P[:], dinv_b)
            nc.sync.dma_start(
                agin2[:, :].rearrange("(a p) b -> p a b", p=128)[:, :, 0:32], HS[:])
            nc.gpsimd.collective_compute(
                "AllGather", ALU.bypass, replica_groups=[list(range(NCORES))],
                ins=[agin2.opt()], outs=[agout2.opt()])
            aggregate(agout2, ACC)
            combine(ACC, P, B2, H)

            # ---- head: global max pool + FC + log_softmax ----
            GMAX = per_p.tile([128, 32], dt.float32)
            nc.vector.tensor_copy(GMAX[:], H[:, 0, :])
            for t in range(1, NB):
                nc.vector.tensor_tensor(GMAX[:], GMAX[:], H[:, t, :], op=ALU.max)
            psg = psd_p.tile([32, 128], dt.float32, tag="ptr")
            nc.tensor.transpose(out=psg[:], in_=GMAX[:], identity=IDN[:])
            GT = per_p.tile([32, 128], dt.float32)
            nc.scalar.copy(GT[:], psg[:])
            GV = per_p.tile([32, 1], dt.float32)
            nc.vector.reduce_max(GV[:], GT[:], axis=AX.X)
            nc.sync.dma_start(arin[:, :], GV[:])
            nc.gpsimd.collective_compute(
                "AllReduce", ALU.max, replica_groups=[list(range(NCORES))],
                ins=[arin.opt()], outs=[arout.opt()])
            GAR = per_p.tile([32, 1], dt.float32)
            nc.sync.dma_start(GAR[:], arout[:, :])
            psl = psd_p.tile([1, 5], dt.float32, tag="plg")
            nc.tensor.matmul(psl[:], lhsT=GAR[:], rhs=FCW[:], start=True, stop=True)
            LG = per_p.tile([1, 5], dt.float32)
            nc.vector.tensor_add(LG[:], psl[:], FCB[:])
            MX = per_p.tile([1, 1], dt.float32)
            nc.vector.reduce_max(MX[:], LG[:], axis=AX.X)
            nc.vector.tensor_tensor(LG[:], LG[:], MX[:].to_broadcast([1, 5]),
                                    op=ALU.subtract)
            EX = per_p.tile([1, 5], dt.float32)
            nc.scalar.activation(EX[:], LG[:], AF.Exp)
            SM = per_p.tile([1, 1], dt.float32)
            nc.vector.reduce_sum(SM[:], EX[:], axis=AX.X)
            LS = per_p.tile([1, 1], dt.float32)
            nc.scalar.activation(LS[:], SM[:], AF.Ln)
            nc.vector.tensor_tensor(LG[:], LG[:], LS[:].to_broadcast([1, 5]),
                                    op=ALU.subtract)
            nc.sync.dma_start(out_t[:, :], LG[:])

    nc.compile()

    in_maps = []
    for k in range(NCORES):
        in_maps.append({
            "xT": xT_all[k], "g16": g16_all[k], "dlf": dlf_all[k], "iota": iota,
            "W1": np.asarray(W1, np.float32), "W2": np.asarray(W2, np.float32),
            "b1t": b1t, "b2t": b2t, "fcW": np.asarray(fcW, np.float32), "fcb": fcb2,
        })
    import os, time as _time
    global RUN_WALL_NS
    trace = os.environ.get("GNN_TRACE", "0") == "1"
    _t0 = _time.time()
    res = run_bass_kernel_spmd(nc, in_maps, core_ids=list(range(NCORES)), trace=trace)
    RUN_WALL_NS = int((_time.time() - _t0) * 1e9)
    LAST_RESULTS = res
    return res.results[0]["out"].astype(np.float32)



# revision 10
# speedup vs baseline: 2.1151x; 2.1151x over previous
"""ChessGNN (2-layer GCN + global max pool + FC + log_softmax) on 8 Trainium2 cores.

v2 strategy (edge-parallel, dst-range sharded, host-folded normalization):

  Math (per GCN layer, self-loops included as ordinary edges):
      out[i] = relu( dinv[i] * sum_{e: dst(e)=i} msg[src(e)] + b )
  L1: msg1[j] = dinv[j] * x[j]            (8-wide);  out needs @W1 post-agg
  L2: msg2[j] = (dinv[j] * h1[j]) @ W2    (32-wide); W2 folded pre-AllGather

  - deg/dinv computed on HOST (edge_index is input data); no device deg pass.
  - Core k owns dst nodes [12500k, 12500(k+1)). Within a core, edges sorted by
    (src-range g of 32768 padded-global rows, dst 64-bucket), cut into chunks
    of <=128 edges; chunk grid (count per (g,bucket)) is max'd over cores so
    the SPMD program is shared; per-core pad lanes get dloc=999 (one-hot row
    of zeros -> no contribution).
  - Gather: GPSIMD dma_gather, 256B rows (128 bf16; cols 0:F used), int16 idx,
    GATHER_CHUNKS chunks (128 idx each) per instruction to amortize the 994ns
    SWDGE fixed cost.  L1 and L2 share one index tensor (same edge order).
  - Segment-sum: per chunk, one-hot [128e, 64slot] built on DVE (is_equal vs
    iota), then PE matmul one-hot^T @ msgs accumulated straight into PSUM.
    PSUM holds the FULL node-slice accumulator across all 4 src ranges
    (banks are pre-zeroed by PE matmuls; accumulation uses start=False):
       L1: S1x [128p, 98slab, 8]  fp32 = 2 banks
       L2: S2  [128p, 98slab, 32] fp32 = 7 banks (16 slabs/bank)
    node n <-> (slab = n>>7, partition = n&127); 64-bucket b -> slab b>>1,
    partition offset 64*(b&1).
  - L1 tail per slab: u = dinv*S1x (DVE) -> PE transpose -> uT [8,128] -> PE
    matmul uT^T@W1 -> relu(dinv*. + dinv*b1) -> hs1 -> PE transpose+matmul W2
    -> t2 slice (cols 0:32 of zeroed 256B-stride staging).
  - L1 gather table is host-built (xs_pad already 256B-stride); the L2 table
    comes from one AllGather of the 256B-stride slices (no DRAM->DRAM
    expands; 16B/64B-descriptor DMAs and >1024-idx gathers crashed HW).
  - Head: per-slab max accum -> transpose -> AllReduce(max) -> FC -> lsm.
"""
import numpy as np

import concourse.bass as bass
import concourse.bacc as bacc
import concourse.mybir as mybir
import concourse.tile as tile
from concourse.bass_utils import run_bass_kernel_spmd
from concourse.masks import make_identity

N = 100000
NCORES = 8
S = N // NCORES            # 12500 nodes per core
NSLAB = 98                 # slabs of 128 dst nodes (98*128 = 12544)
SP = NSLAB * 128           # padded slice rows
NPAD = NCORES * SP         # padded global rows = 100352
RNG = 32768                # int16 gather range (rows)
NRANGES = (NPAD + RNG - 1) // RNG  # 4
NBUCK = 64                 # dst bucket size (one-hot width)
NB = SP // NBUCK           # buckets per core slice = 196
GATHER_CHUNKS = 8          # chunks (=128 idx) per dma_gather instruction (1024 idx, HW-proven)
PADDLOC = 999.0
ROWB = 128                 # gather-table row, bf16 elems (256B)

LAST_RESULTS = None
RUN_WALL_NS = None


def _marshal(x, edge_index, W1, b1, W2, b2, fcW, fcb):
    """Host-side sharding/precompute. Returns per-core input dicts + schedule."""
    x = np.asarray(x, np.float32)
    ei = np.asarray(edge_index)
    src = ei[0].astype(np.int64)
    dst = ei[1].astype(np.int64)

    deg = np.bincount(dst, minlength=N).astype(np.float32) + 1.0  # self loop
    dinv = 1.0 / np.sqrt(deg)

    # self loops are ordinary edges under the folded normalization
    loop = np.arange(N, dtype=np.int64)
    src2 = np.concatenate([src, loop])
    dst2 = np.concatenate([dst, loop])

    gid = (src2 // S) * SP + (src2 % S)          # padded global row of src
    owner = dst2 // S

    # ---- per-core edge lists sorted by (range, bucket) ----
    per = []
    cnt = np.zeros((NCORES, NRANGES, NB), np.int64)
    for k in range(NCORES):
        m = owner == k
        sg = gid[m]
        dl = dst2[m] - k * S                      # local node id 0..12499
        rg = sg >> 15
        bb = dl // NBUCK
        order = np.lexsort((bb, rg))
        sg, dl, rg, bb = sg[order], dl[order], rg[order], bb[order]
        np.add.at(cnt[k], (rg, bb), 1)
        per.append((sg, dl, rg, bb))

    chunks = (cnt.max(axis=0) + 127) // 128       # [NRANGES, NB]
    cpr = chunks.sum(axis=1)                      # chunks per range
    C = int(cpr.sum())

    # gather instruction schedule per range: full GATHER_CHUNKS + tail
    # sched[g] = list of chunk-counts per instruction
    sched = []
    for g in range(NRANGES):
        n = int(cpr[g])
        instr = [GATHER_CHUNKS] * (n // GATHER_CHUNKS)
        if n % GATHER_CHUNKS:
            instr.append(n % GATHER_CHUNKS)
        sched.append(instr)
    tot_idx = C * 128

    # chunk meta in emission order: (g, bucket)
    chunk_meta = []
    for g in range(NRANGES):
        for b in range(NB):
            chunk_meta.extend([(g, b)] * int(chunks[g, b]))
    assert len(chunk_meta) == C

    # ---- per-core gather idx + dloc arrays on the shared chunk grid ----
    g16_all, dlf_all = [], []
    for k in range(NCORES):
        sg, dl, rg, bb = per[k]
        gidx = np.zeros(C * 128, np.int16)
        dloc = np.full(C * 128, PADDLOC, np.float32)
        pos = 0
        ptr = 0
        for g in range(NRANGES):
            for b in range(NB):
                n = int(cnt[k, g, b])
                cap = int(chunks[g, b]) * 128
                sl = slice(ptr, ptr + n)
                gidx[pos:pos + n] = (sg[sl] - g * RNG).astype(np.int16)
                dloc[pos:pos + n] = (dl[sl] - (b * NBUCK)).astype(np.float32)
                ptr += n
                pos += cap
        # idx slabs per instruction: idx i -> [i%16, i//16], replicated to 128p
        slabs = []
        off = 0
        for g in range(NRANGES):
            for ni in sched[g]:
                nidx = ni * 128
                blk = gidx[off:off + nidx].reshape(nidx // 16, 16).T  # [16, n/16]
                slabs.append(blk)
                off += nidx
        flat = np.concatenate(slabs, axis=1)                 # [16, tot_idx/16]
        g16 = np.tile(flat, (8, 1))                          # [128, tot_idx/16]
        dlf = dloc.reshape(C, 128).T.astype(np.float32)      # [128, C] (cast below)
        g16_all.append(np.ascontiguousarray(g16))
        dlf_all.append(_to_bf16(dlf))

    # ---- per-core dense inputs ----
    xs = x * dinv[:, None]                                   # L1 messages, [N, 8]
    xs_pad = np.zeros((NPAD, ROWB), np.float32)              # 256B-stride rows
    for k in range(NCORES):
        xs_pad[k * SP:k * SP + S, 0:8] = xs[k * S:(k + 1) * S]

    dinv_t_all, dinv_b1_all, b2m_all = [], [], []
    b1 = np.asarray(b1, np.float32)
    b2 = np.asarray(b2, np.float32)
    for k in range(NCORES):
        dv = np.zeros(SP, np.float32)
        dv[:S] = dinv[k * S:(k + 1) * S]
        dinv_t = dv.reshape(NSLAB, 128).T.copy()             # [128, NSLAB]
        # L1 bias term: relu(dinv*P + dinv*b1) -> bias = dinv_i * b1_f
        db1 = dv[:, None] * b1[None, :]                      # [SP, 32]
        db1 = db1.reshape(NSLAB, 128, 32).transpose(1, 0, 2) # [128, NSLAB, 32]
        # L2 bias with -inf on pad rows so pad h2 never wins the max pool
        b2m = np.tile(b2[None, :], (SP, 1))
        b2m[S:] = -1e30
        b2m = b2m.reshape(NSLAB, 128, 32).transpose(1, 0, 2)
        dinv_t_all.append(dinv_t)
        dinv_b1_all.append(db1.astype(np.float32))
        b2m_all.append(b2m.astype(np.float32))

    iota = np.tile(np.arange(NBUCK, dtype=np.float32), (128, 1))  # [128, NBUCK]

    host = {
        "xs_pad": _to_bf16(xs_pad),
        "iota": _to_bf16(iota),
        "W1": _to_bf16(np.asarray(W1, np.float32)),
        "W2": _to_bf16(np.asarray(W2, np.float32)),
        "fcW": np.asarray(fcW, np.float32),
        "fcb": np.asarray(fcb, np.float32)[None, :],
    }
    percore = []
    for k in range(NCORES):
        percore.append({
            "g16": g16_all[k], "dlf": dlf_all[k],
            "dinv_t": dinv_t_all[k], "db1": dinv_b1_all[k], "b2m": b2m_all[k],
        })
    return host, percore, chunks, sched, chunk_meta, C


def _to_bf16(a):
    import ml_dtypes
    return a.astype(ml_dtypes.bfloat16)


def build(x, edge_index, W1, b1, W2, b2, fcW, fcb):
    host, percore, chunks, sched, chunk_meta, C = _marshal(
        x, edge_index, W1, b1, W2, b2, fcW, fcb)

    tot_idx = C * 128
    nc = bacc.Bacc("TRN2", target_bir_lowering=False, debug=False,
                   num_devices=NCORES)
    dt = mybir.dt
    AF = mybir.ActivationFunctionType
    ALU = mybir.AluOpType
    AX = mybir.AxisListType

    xs_t = nc.dram_tensor("xs_pad", [NPAD, ROWB], dt.bfloat16, kind="ExternalInput")
    g16_t = nc.dram_tensor("g16", [128, tot_idx // 16], dt.int16, kind="ExternalInput")
    dlf_t = nc.dram_tensor("dlf", [128, C], dt.bfloat16, kind="ExternalInput")
    iota_t = nc.dram_tensor("iota", [128, NBUCK], dt.bfloat16, kind="ExternalInput")
    dinv_t_t = nc.dram_tensor("dinv_t", [128, NSLAB], dt.float32, kind="ExternalInput")
    db1_t = nc.dram_tensor("db1", [128, NSLAB, 32], dt.float32, kind="ExternalInput")
    b2m_t = nc.dram_tensor("b2m", [128, NSLAB, 32], dt.float32, kind="ExternalInput")
    W1_t = nc.dram_tensor("W1", [8, 32], dt.bfloat16, kind="ExternalInput")
    W2_t = nc.dram_tensor("W2", [32, 32], dt.bfloat16, kind="ExternalInput")
    fcW_t = nc.dram_tensor("fcW", [32, 5], dt.float32, kind="ExternalInput")
    fcb_t = nc.dram_tensor("fcb", [1, 5], dt.float32, kind="ExternalInput")
    out_t = nc.dram_tensor("out", [1, 5], dt.float32, kind="ExternalOutput")

    with tile.TileContext(nc) as tc:
        with (
            tc.tile_pool(name="per", bufs=1) as per_p,
            tc.tile_pool(name="gt", bufs=3) as gt_p,
            tc.tile_pool(name="oh", bufs=4) as oh_p,
            tc.tile_pool(name="sl", bufs=3) as sl_p,       # small per-slab tiles
            tc.tile_pool(name="psS", bufs=1, space="PSUM") as psS_p,
            tc.tile_pool(name="psx", bufs=1, space="PSUM") as psx_p,
            tc.tile_pool(name="dram", bufs=1, space="DRAM") as dram_p,
        ):
            # ---- persistent SBUF ----
            G16 = per_p.tile([128, tot_idx // 16], dt.int16)
            DLF = per_p.tile([128, C], dt.bfloat16)
            IO = per_p.tile([128, NBUCK], dt.bfloat16)
            DINV = per_p.tile([128, NSLAB], dt.float32)
            DB1 = per_p.tile([128, NSLAB, 32], dt.float32)
            B2M = per_p.tile([128, NSLAB, 32], dt.float32)
            W1s = per_p.tile([8, 32], dt.bfloat16)
            W2s = per_p.tile([32, 32], dt.bfloat16)
            FCW = per_p.tile([32, 5], dt.float32)
            FCB = per_p.tile([1, 5], dt.float32)
            IDN = per_p.tile([128, 128], dt.float32)
            Z1 = per_p.tile([1, 128], dt.bfloat16)           # zeroing matmul lhsT
            Z512 = per_p.tile([1, 512], dt.bfloat16)         # zeroing matmul rhs
            GMAX = per_p.tile([128, 32], dt.float32)
            T2S = per_p.tile([128, NSLAB, ROWB], dt.bfloat16)  # staged t2 slice

            for t_, s_ in ((G16, g16_t), (DLF, dlf_t), (IO, iota_t),
                           (DINV, dinv_t_t), (DB1, db1_t), (B2M, b2m_t),
                           (W1s, W1_t), (W2s, W2_t), (FCW, fcW_t), (FCB, fcb_t)):
                nc.sync.dma_start(t_[:], s_[:])
            nc.vector.memset(Z1[:], 0.0)
            nc.vector.memset(Z512[:], 0.0)
            make_identity(nc, IDN[:])

            # ---- DRAM ----
            agin = dram_p.tile([SP, ROWB], dt.bfloat16)
            agout = dram_p.tile([NPAD, ROWB], dt.bfloat16)    # L2 gather table
            arin = dram_p.tile([32, 1], dt.float32)
            arout = dram_p.tile([32, 1], dt.float32)

            # PSUM accumulators: 7 raw banks [128, 512] fp32
            SB = [psS_p.tile([128, 512], dt.float32, tag=f"bank{i}",
                             name=f"sbank{i}")
                  for i in range(7)]
            SCR = psx_p.tile([128, 512], dt.float32, tag="scratch")

            # manual sub-bank views for the L1 per-slab chain (banks 2..4
            # only become S2 accumulators in layer 2)
            def v128(i):
                return SB[i][:].rearrange("p (a f) -> p a f", f=128)

            def v32(i):
                return SB[i][:].rearrange("p (a f) -> p a f", f=32)

            def psum_zero(banks):
                for i in banks:
                    nc.tensor.matmul(SB[i][:], lhsT=Z1[:], rhs=Z512[:],
                                     start=True, stop=True)

            def s1x_ap(b):
                """L1 accumulator slot for 64-bucket b: [64, 8] fp32."""
                slab, v = b >> 1, b & 1
                bank, col = slab >> 6, slab & 63
                ap = SB[bank][:].rearrange("p (a f) -> p a f", f=8)
                return ap[64 * v:64 * (v + 1), col, :]

            def s1x_slab(sl):
                ap = SB[sl >> 6][:].rearrange("p (a f) -> p a f", f=8)
                return ap[:, sl & 63, :]

            def s2_ap(b):
                """L2 accumulator slot for 64-bucket b: [64, 32] fp32."""
                slab, v = b >> 1, b & 1
                bank, col = slab >> 4, slab & 15
                ap = SB[bank][:].rearrange("p (a f) -> p a f", f=32)
                return ap[64 * v:64 * (v + 1), col, :]

            def s2_slab(sl):
                ap = SB[sl >> 4][:].rearrange("p (a f) -> p a f", f=32)
                return ap[:, sl & 15, :]

            # ---- shared aggregation loop ----
            def aggregate(table, width, slot_ap):
                """Gather+one-hot+matmul over all chunks; accumulate in PSUM."""
                ci = 0        # chunk index in grid order
                ioff = 0      # idx offset (units of 16 cols in G16)
                for g in range(NRANGES):
                    r0, r1 = g * RNG, min((g + 1) * RNG, NPAD)
                    src_ap = table[r0:r1, :]
                    for ii, nch in enumerate(sched_g[g]):
                        nidx = nch * 128
                        gt = gt_p.tile([128, GATHER_CHUNKS, ROWB], dt.bfloat16,
                                       tag="gt")
                        nc.gpsimd.dma_gather(
                            gt[:, 0:nch, :], src_ap,
                            G16[:, ioff:ioff + nidx // 16], nidx, nidx, ROWB)
                        ioff += nidx // 16
                        for m0 in range(0, nch, 8):
                            mn = min(8, nch - m0)
                            oh = oh_p.tile([128, 8, NBUCK], dt.bfloat16, tag="oh")
                            j0 = ci + m0
                            nc.vector.tensor_tensor(
                                out=oh[:, 0:mn, :],
                                in0=DLF[:, j0:j0 + mn]
                                    .rearrange("p (c o) -> p c o", o=1)
                                    .to_broadcast([128, mn, NBUCK]),
                                in1=IO[:].rearrange("p (o s) -> p o s", o=1)
                                    .to_broadcast([128, mn, NBUCK]),
                                op=ALU.is_equal)
                            for s in range(mn):
                                b = chunk_meta[j0 + s][1]
                                nc.tensor.matmul(
                                    slot_ap(b), lhsT=oh[:, s, :],
                                    rhs=gt[:, m0 + s, 0:width],
                                    start=False, stop=True,
                                    skip_group_check=True)
                        ci += nch

            sched_g = sched

            # =========== LAYER 1 ===========
            # xs_pad is already the 256B-stride gather table (host-built)
            nc.vector.memset(T2S[:], 0.0)
            psum_zero([0, 1])
            aggregate(xs_t, 8, s1x_ap)

            # per-slab tail: u=dinv*S1x -> uT -> @W1 -> hs1 -> @W2 -> t2 slice
            for sl in range(NSLAB):
                par = sl & 1
                u = sl_p.tile([128, 8], dt.float32, tag="u")
                nc.vector.tensor_scalar_mul(u[:], s1x_slab(sl),
                                            DINV[:, sl:sl + 1])
                put = v128(2)[0:8, par, :]
                nc.tensor.transpose(out=put, in_=u[:], identity=IDN[:])
                uT = sl_p.tile([8, 128], dt.bfloat16, tag="uT")
                nc.scalar.copy(uT[:], put)
                ph = v32(4)[:, par, :]
                nc.tensor.matmul(ph, lhsT=uT[:], rhs=W1s[:],
                                 start=True, stop=True)
                # hs1 = relu(dinv*P + dinv*b1)
                hp = sl_p.tile([128, 32], dt.float32, tag="hp")
                nc.vector.scalar_tensor_tensor(
                    hp[:], ph, DINV[:, sl:sl + 1], DB1[:, sl, :],
                    op0=ALU.mult, op1=ALU.add)
                hs1 = sl_p.tile([128, 32], dt.float32, tag="hs1")
                nc.scalar.activation(hs1[:], hp[:], AF.Relu)
                # t2 = hs1 @ W2
                pt = v128(3)[0:32, par, :]
                nc.tensor.transpose(out=pt, in_=hs1[:], identity=IDN[:])
                hT = sl_p.tile([32, 128], dt.bfloat16, tag="hT")
                nc.scalar.copy(hT[:], pt)
                p2 = v32(4)[:, 2 + par, :]
                nc.tensor.matmul(p2, lhsT=hT[:], rhs=W2s[:],
                                 start=True, stop=True)
                nc.scalar.copy(T2S[:, sl, 0:32], p2)

            # slice (full 256B-stride rows) -> DRAM -> AllGather
            nc.sync.dma_start(
                agin[:, :].rearrange("(a p) b -> p a b", p=128), T2S[:])
            nc.gpsimd.collective_compute(
                "AllGather", ALU.bypass, replica_groups=[list(range(NCORES))],
                ins=[agin.opt()], outs=[agout.opt()])

            # =========== LAYER 2 ===========
            psum_zero(range(7))
            aggregate(agout, 32, s2_ap)

            # combine + max pool, batched per bank
            nc.vector.memset(GMAX[:], -1e30)
            for sl in range(NSLAB):
                h2p = sl_p.tile([128, 32], dt.float32, tag="h2p")
                nc.vector.scalar_tensor_tensor(
                    h2p[:], s2_slab(sl), DINV[:, sl:sl + 1], B2M[:, sl, :],
                    op0=ALU.mult, op1=ALU.add)
                h2 = sl_p.tile([128, 32], dt.float32, tag="h2")
                nc.scalar.activation(h2[:], h2p[:], AF.Relu)
                nc.vector.tensor_tensor(GMAX[:], GMAX[:], h2[:], op=ALU.max)

            # ---- head ----
            psg = SCR[:].rearrange("p (a f) -> p a f", f=128)[0:32, 0, :]
            nc.tensor.transpose(out=psg, in_=GMAX[:], identity=IDN[:])
            GT = per_p.tile([32, 128], dt.float32)
            nc.scalar.copy(GT[:], psg)
            GV = per_p.tile([32, 1], dt.float32)
            nc.vector.reduce_max(GV[:], GT[:], axis=AX.X)
            nc.sync.dma_start(arin[:, :], GV[:])
            nc.gpsimd.collective_compute(
                "AllReduce", ALU.max, replica_groups=[list(range(NCORES))],
                ins=[arin.opt()], outs=[arout.opt()])
            GAR = per_p.tile([32, 1], dt.float32)
            nc.sync.dma_start(GAR[:], arout[:, :])
            psl = SCR[:].rearrange("p (a f) -> p a f", f=32)[0:1, 15, 0:5]
            nc.tensor.matmul(psl, lhsT=GAR[:], rhs=FCW[:], start=True, stop=True)
            LG = per_p.tile([1, 5], dt.float32)
            nc.vector.tensor_add(LG[:], psl, FCB[:])
            MX = per_p.tile([1, 1], dt.float32)
            nc.vector.reduce_max(MX[:], LG[:], axis=AX.X)
            nc.vector.tensor_tensor(LG[:], LG[:], MX[:].to_broadcast([1, 5]),
                                    op=ALU.subtract)
            EX = per_p.tile([1, 5], dt.float32)
            nc.scalar.activation(EX[:], LG[:], AF.Exp)
            SM = per_p.tile([1, 1], dt.float32)
            nc.vector.reduce_sum(SM[:], EX[:], axis=AX.X)
            LS = per_p.tile([1, 1], dt.float32)
            nc.scalar.activation(LS[:], SM[:], AF.Ln)
            nc.vector.tensor_tensor(LG[:], LG[:], LS[:].to_broadcast([1, 5]),
                                    op=ALU.subtract)
            nc.sync.dma_start(out_t[:, :], LG[:])

    in_maps = []
    for k in range(NCORES):
        m = {"xs_pad": host["xs_pad"], "g16": percore[k]["g16"],
             "dlf": percore[k]["dlf"], "iota": host["iota"],
             "dinv_t": percore[k]["dinv_t"], "db1": percore[k]["db1"],
             "b2m": percore[k]["b2m"], "W1": host["W1"], "W2": host["W2"],
             "fcW": host["fcW"], "fcb": host["fcb"]}
        in_maps.append(m)
    return nc, in_maps


def kernel(x, edge_index, W1, b1, W2, b2, fcW, fcb):
    global LAST_RESULTS, RUN_WALL_NS
    nc, in_maps = build(x, edge_index, W1, b1, W2, b2, fcW, fcb)
    nc.compile()
    import os, time as _time
    trace = os.environ.get("GNN_TRACE", "0") == "1"
    _t0 = _time.time()
    res = run_bass_kernel_spmd(nc, in_maps, core_ids=list(range(NCORES)),
                               trace=trace)
    RUN_WALL_NS = int((_time.time() - _t0) * 1e9)
    LAST_RESULTS = res
    return res.results[0]["out"].astype(np.float32)


# revision 11
# speedup vs baseline: 2.7600x; 1.3049x over previous
"""ChessGNN (2-layer GCN + global max pool + FC + log_softmax) on 8 Trainium2 cores.

v2 strategy (edge-parallel, dst-range sharded, host-folded normalization):

  Math (per GCN layer, self-loops included as ordinary edges):
      out[i] = relu( dinv[i] * sum_{e: dst(e)=i} msg[src(e)] + b )
  L1: msg1[j] = dinv[j] * x[j]            (8-wide);  out needs @W1 post-agg
  L2: msg2[j] = (dinv[j] * h1[j]) @ W2    (32-wide); W2 folded pre-AllGather

  - deg/dinv computed on HOST (edge_index is input data); no device deg pass.
  - Core k owns dst nodes [12500k, 12500(k+1)). Within a core, edges sorted by
    (src-range g of 32768 padded-global rows, dst 64-bucket), cut into chunks
    of <=128 edges; chunk grid (count per (g,bucket)) is max'd over cores so
    the SPMD program is shared; per-core pad lanes get dloc=999 (one-hot row
    of zeros -> no contribution).
  - Gather: GPSIMD dma_gather, 256B rows (128 bf16; cols 0:F used), int16 idx,
    GATHER_CHUNKS chunks (128 idx each) per instruction to amortize the 994ns
    SWDGE fixed cost.  L1 and L2 share one index tensor (same edge order).
  - Segment-sum: per chunk, one-hot [128e, 64slot] built on DVE (is_equal vs
    iota), then PE matmul one-hot^T @ msgs accumulated straight into PSUM.
    PSUM holds the FULL node-slice accumulator across all 4 src ranges
    (banks are pre-zeroed by PE matmuls; accumulation uses start=False):
       L1: S1x [128p, 98slab, 8]  fp32 = 2 banks
       L2: S2  [128p, 98slab, 32] fp32 = 7 banks (16 slabs/bank)
    node n <-> (slab = n>>7, partition = n&127); 64-bucket b -> slab b>>1,
    partition offset 64*(b&1).
  - L1 tail per slab: u = dinv*S1x (DVE) -> PE transpose -> uT [8,128] -> PE
    matmul uT^T@W1 -> relu(dinv*. + dinv*b1) -> hs1 -> PE transpose+matmul W2
    -> t2 slice (cols 0:32 of zeroed 256B-stride staging).
  - L1 gather table is host-built (xs_pad already 256B-stride); the L2 table
    comes from one AllGather of the 256B-stride slices (no DRAM->DRAM
    expands; 16B/64B-descriptor DMAs and >1024-idx gathers crashed HW).
  - Head: per-slab max accum -> transpose -> AllReduce(max) -> FC -> lsm.
"""
import numpy as np

import concourse.bass as bass
import concourse.bacc as bacc
import concourse.mybir as mybir
import concourse.tile as tile
from concourse.bass_utils import run_bass_kernel_spmd
from concourse.masks import make_identity

N = 100000
NCORES = 8
S = N // NCORES            # 12500 nodes per core
NSLAB = 98                 # slabs of 128 dst nodes (98*128 = 12544)
SP = NSLAB * 128           # padded slice rows
NPAD = NCORES * SP         # padded global rows = 100352
RNG = 32768                # int16 gather range (rows)
NRANGES = (NPAD + RNG - 1) // RNG  # 4
NBUCK = 64                 # dst bucket size (one-hot width)
NB = SP // NBUCK           # buckets per core slice = 196
GATHER_CHUNKS = 8          # chunks (=128 idx) per dma_gather instruction (1024 idx, HW-proven)
PADDLOC = 999.0
ROWB = 128                 # gather-table row, bf16 elems (256B)

LAST_RESULTS = None
RUN_WALL_NS = None


def _dma_gather_64b(nc, out_ap, in_ap, idxs_ap, num_idxs, elem_size):
    """dma_gather with elem_size_bytes below 256 (non-transpose HBM path).

    The ucode uses elem_size_bytes directly as the descriptor length; bass's
    256B assert is a transpose-path restriction.  HW-verified for 1024 idx,
    elem 32 bf16, 256B row stride."""
    import concourse.ap_utils as ap_utils
    from concourse._compat import exact_div
    gp = nc.gpsimd
    assert idxs_ap.dtype == mybir.dt.int16
    assert in_ap.dtype == out_ap.dtype
    elem_step = in_ap.ap[0][0]
    assert ap_utils.ap_is_contiguous(out_ap.ap[1:])
    assert ap_utils.ap_is_contiguous(idxs_ap.ap[1:])
    assert in_ap.ap[-1][1] == out_ap.ap[-1][1] == elem_size
    stride_bytes_256 = exact_div(elem_step * mybir.dt.size(in_ap.dtype), 256)
    return gp.add_instruction(
        mybir.InstDMAGatherAnt(
            name=gp.bass.get_next_instruction_name(),
            ins=[*gp.lower_ap_dma(in_ap, for_custom_bir_dma=True),
                 gp.lower_ap(idxs_ap),
                 gp.lower_val_access(gp.to_reg(num_idxs))],
            outs=[gp.lower_ap(out_ap)],
            transpose=False, num_idxs=num_idxs, elem_size=elem_size,
            stride_bytes_256=stride_bytes_256, gen_mode=0, single_packet=True,
            queue_num=0, sbuf_tokens_per_rank=0, sbuf_free_dim_per_rank=0,
            sbuf_free_dim_pad_per_rank=0, sbuf_byte_offset=0))


def _marshal(x, edge_index, W1, b1, W2, b2, fcW, fcb):
    """Host-side sharding/precompute. Returns per-core input dicts + schedule."""
    x = np.asarray(x, np.float32)
    ei = np.asarray(edge_index)
    src = ei[0].astype(np.int64)
    dst = ei[1].astype(np.int64)

    deg = np.bincount(dst, minlength=N).astype(np.float32) + 1.0  # self loop
    dinv = 1.0 / np.sqrt(deg)

    # self loops are ordinary edges under the folded normalization
    loop = np.arange(N, dtype=np.int64)
    src2 = np.concatenate([src, loop])
    dst2 = np.concatenate([dst, loop])

    gid = (src2 // S) * SP + (src2 % S)          # padded global row of src
    owner = dst2 // S

    # ---- per-core edge lists sorted by (range, bucket) ----
    per = []
    cnt = np.zeros((NCORES, NRANGES, NB), np.int64)
    for k in range(NCORES):
        m = owner == k
        sg = gid[m]
        dl = dst2[m] - k * S                      # local node id 0..12499
        rg = sg >> 15
        bb = dl // NBUCK
        order = np.lexsort((bb, rg))
        sg, dl, rg, bb = sg[order], dl[order], rg[order], bb[order]
        np.add.at(cnt[k], (rg, bb), 1)
        per.append((sg, dl, rg, bb))

    chunks = (cnt.max(axis=0) + 127) // 128       # [NRANGES, NB]
    cpr = chunks.sum(axis=1)                      # chunks per range
    C = int(cpr.sum())

    # gather instruction schedule per range: full GATHER_CHUNKS + tail
    # sched[g] = list of chunk-counts per instruction
    sched = []
    for g in range(NRANGES):
        n = int(cpr[g])
        instr = [GATHER_CHUNKS] * (n // GATHER_CHUNKS)
        if n % GATHER_CHUNKS:
            instr.append(n % GATHER_CHUNKS)
        sched.append(instr)
    tot_idx = C * 128

    # chunk meta in emission order: (g, bucket)
    chunk_meta = []
    for g in range(NRANGES):
        for b in range(NB):
            chunk_meta.extend([(g, b)] * int(chunks[g, b]))
    assert len(chunk_meta) == C

    # ---- per-core gather idx + dloc arrays on the shared chunk grid ----
    g16_all, dlf_all = [], []
    for k in range(NCORES):
        sg, dl, rg, bb = per[k]
        gidx = np.zeros(C * 128, np.int16)
        dloc = np.full(C * 128, PADDLOC, np.float32)
        pos = 0
        ptr = 0
        for g in range(NRANGES):
            for b in range(NB):
                n = int(cnt[k, g, b])
                cap = int(chunks[g, b]) * 128
                sl = slice(ptr, ptr + n)
                gidx[pos:pos + n] = (sg[sl] - g * RNG).astype(np.int16)
                dloc[pos:pos + n] = (dl[sl] - (b * NBUCK)).astype(np.float32)
                ptr += n
                pos += cap
        # idx slabs per instruction: idx i -> [i%16, i//16], replicated to 128p
        slabs = []
        off = 0
        for g in range(NRANGES):
            for ni in sched[g]:
                nidx = ni * 128
                blk = gidx[off:off + nidx].reshape(nidx // 16, 16).T  # [16, n/16]
                slabs.append(blk)
                off += nidx
        flat = np.concatenate(slabs, axis=1)                 # [16, tot_idx/16]
        g16 = np.tile(flat, (8, 1))                          # [128, tot_idx/16]
        dlf = dloc.reshape(C, 128).T.astype(np.float32)      # [128, C] (cast below)
        g16_all.append(np.ascontiguousarray(g16))
        dlf_all.append(_to_bf16(dlf))

    # ---- per-core dense inputs ----
    xs = x * dinv[:, None]                                   # L1 messages, [N, 8]
    xs_pad = np.zeros((NPAD, ROWB), np.float32)              # 256B-stride rows
    for k in range(NCORES):
        xs_pad[k * SP:k * SP + S, 0:8] = xs[k * S:(k + 1) * S]

    dinv_t_all, dinv_b1_all, b2m_all = [], [], []
    b1 = np.asarray(b1, np.float32)
    b2 = np.asarray(b2, np.float32)
    for k in range(NCORES):
        dv = np.zeros(SP, np.float32)
        dv[:S] = dinv[k * S:(k + 1) * S]
        dinv_t = dv.reshape(NSLAB, 128).T.copy()             # [128, NSLAB]
        # L1 bias term: relu(dinv*P + dinv*b1) -> bias = dinv_i * b1_f
        db1 = dv[:, None] * b1[None, :]                      # [SP, 32]
        db1 = db1.reshape(NSLAB, 128, 32).transpose(1, 0, 2) # [128, NSLAB, 32]
        # L2 bias with -inf on pad rows so pad h2 never wins the max pool
        b2m = np.tile(b2[None, :], (SP, 1))
        b2m[S:] = -1e30
        b2m = b2m.reshape(NSLAB, 128, 32).transpose(1, 0, 2)
        dinv_t_all.append(dinv_t)
        dinv_b1_all.append(db1.astype(np.float32))
        b2m_all.append(b2m.astype(np.float32))

    iota = np.tile(np.arange(NBUCK, dtype=np.float32), (128, 1))  # [128, NBUCK]

    host = {
        "xs_pad": _to_bf16(xs_pad),
        "iota": _to_bf16(iota),
        "W1": _to_bf16(np.asarray(W1, np.float32)),
        "W2": _to_bf16(np.asarray(W2, np.float32)),
        "fcW": np.asarray(fcW, np.float32),
        "fcb": np.asarray(fcb, np.float32)[None, :],
    }
    percore = []
    for k in range(NCORES):
        percore.append({
            "g16": g16_all[k], "dlf": dlf_all[k],
            "dinv_t": dinv_t_all[k], "db1": dinv_b1_all[k], "b2m": b2m_all[k],
        })
    return host, percore, chunks, sched, chunk_meta, C


def _to_bf16(a):
    import ml_dtypes
    return a.astype(ml_dtypes.bfloat16)


def build(x, edge_index, W1, b1, W2, b2, fcW, fcb):
    host, percore, chunks, sched, chunk_meta, C = _marshal(
        x, edge_index, W1, b1, W2, b2, fcW, fcb)

    tot_idx = C * 128
    nc = bacc.Bacc("TRN2", target_bir_lowering=False, debug=False,
                   num_devices=NCORES)
    dt = mybir.dt
    AF = mybir.ActivationFunctionType
    ALU = mybir.AluOpType
    AX = mybir.AxisListType

    xs_t = nc.dram_tensor("xs_pad", [NPAD, ROWB], dt.bfloat16, kind="ExternalInput")
    g16_t = nc.dram_tensor("g16", [128, tot_idx // 16], dt.int16, kind="ExternalInput")
    dlf_t = nc.dram_tensor("dlf", [128, C], dt.bfloat16, kind="ExternalInput")
    iota_t = nc.dram_tensor("iota", [128, NBUCK], dt.bfloat16, kind="ExternalInput")
    dinv_t_t = nc.dram_tensor("dinv_t", [128, NSLAB], dt.float32, kind="ExternalInput")
    db1_t = nc.dram_tensor("db1", [128, NSLAB, 32], dt.float32, kind="ExternalInput")
    b2m_t = nc.dram_tensor("b2m", [128, NSLAB, 32], dt.float32, kind="ExternalInput")
    W1_t = nc.dram_tensor("W1", [8, 32], dt.bfloat16, kind="ExternalInput")
    W2_t = nc.dram_tensor("W2", [32, 32], dt.bfloat16, kind="ExternalInput")
    fcW_t = nc.dram_tensor("fcW", [32, 5], dt.float32, kind="ExternalInput")
    fcb_t = nc.dram_tensor("fcb", [1, 5], dt.float32, kind="ExternalInput")
    out_t = nc.dram_tensor("out", [1, 5], dt.float32, kind="ExternalOutput")

    with tile.TileContext(nc) as tc:
        with (
            tc.tile_pool(name="per", bufs=1) as per_p,
            tc.tile_pool(name="gt", bufs=3) as gt_p,
            tc.tile_pool(name="oh", bufs=4) as oh_p,
            tc.tile_pool(name="sl", bufs=3) as sl_p,       # small per-slab tiles
            tc.tile_pool(name="psS", bufs=1, space="PSUM") as psS_p,
            tc.tile_pool(name="psx", bufs=1, space="PSUM") as psx_p,
            tc.tile_pool(name="dram", bufs=1, space="DRAM") as dram_p,
        ):
            # ---- persistent SBUF ----
            G16 = per_p.tile([128, tot_idx // 16], dt.int16)
            DLF = per_p.tile([128, C], dt.bfloat16)
            IO = per_p.tile([128, NBUCK], dt.bfloat16)
            DINV = per_p.tile([128, NSLAB], dt.float32)
            DB1 = per_p.tile([128, NSLAB, 32], dt.float32)
            B2M = per_p.tile([128, NSLAB, 32], dt.float32)
            W1s = per_p.tile([8, 32], dt.bfloat16)
            W2s = per_p.tile([32, 32], dt.bfloat16)
            FCW = per_p.tile([32, 5], dt.float32)
            FCB = per_p.tile([1, 5], dt.float32)
            IDN = per_p.tile([128, 128], dt.float32)
            Z1 = per_p.tile([1, 128], dt.bfloat16)           # zeroing matmul lhsT
            Z512 = per_p.tile([1, 512], dt.bfloat16)         # zeroing matmul rhs
            GMAX = per_p.tile([128, 32], dt.float32)
            T2S = per_p.tile([128, NSLAB, ROWB], dt.bfloat16)  # staged t2 slice

            for t_, s_ in ((G16, g16_t), (DLF, dlf_t), (IO, iota_t),
                           (DINV, dinv_t_t), (DB1, db1_t), (B2M, b2m_t),
                           (W1s, W1_t), (W2s, W2_t), (FCW, fcW_t), (FCB, fcb_t)):
                nc.sync.dma_start(t_[:], s_[:])
            nc.vector.memset(Z1[:], 0.0)
            nc.vector.memset(Z512[:], 0.0)
            make_identity(nc, IDN[:])

            # ---- DRAM ----
            agin = dram_p.tile([SP, ROWB], dt.bfloat16)
            agout = dram_p.tile([NPAD, ROWB], dt.bfloat16)    # L2 gather table
            arin = dram_p.tile([32, 1], dt.float32)
            arout = dram_p.tile([32, 1], dt.float32)

            # PSUM accumulators: 7 raw banks [128, 512] fp32
            SB = [psS_p.tile([128, 512], dt.float32, tag=f"bank{i}",
                             name=f"sbank{i}")
                  for i in range(7)]
            SCR = psx_p.tile([128, 512], dt.float32, tag="scratch")

            # manual sub-bank views for the L1 per-slab chain (banks 2..4
            # only become S2 accumulators in layer 2)
            def v128(i):
                return SB[i][:].rearrange("p (a f) -> p a f", f=128)

            def v32(i):
                return SB[i][:].rearrange("p (a f) -> p a f", f=32)

            def psum_zero(banks):
                for i in banks:
                    nc.tensor.matmul(SB[i][:], lhsT=Z1[:], rhs=Z512[:],
                                     start=True, stop=True)

            def s1x_ap(b):
                """L1 accumulator slot for 64-bucket b: [64, 8] fp32."""
                slab, v = b >> 1, b & 1
                bank, col = slab >> 6, slab & 63
                ap = SB[bank][:].rearrange("p (a f) -> p a f", f=8)
                return ap[64 * v:64 * (v + 1), col, :]

            def s1x_slab(sl):
                ap = SB[sl >> 6][:].rearrange("p (a f) -> p a f", f=8)
                return ap[:, sl & 63, :]

            def s2_ap(b):
                """L2 accumulator slot for 64-bucket b: [64, 32] fp32."""
                slab, v = b >> 1, b & 1
                bank, col = slab >> 4, slab & 15
                ap = SB[bank][:].rearrange("p (a f) -> p a f", f=32)
                return ap[64 * v:64 * (v + 1), col, :]

            def s2_slab(sl):
                ap = SB[sl >> 4][:].rearrange("p (a f) -> p a f", f=32)
                return ap[:, sl & 15, :]

            # ---- shared aggregation loop ----
            def aggregate(table, width, slot_ap):
                """Gather+one-hot+matmul over all chunks; accumulate in PSUM."""
                ci = 0        # chunk index in grid order
                ioff = 0      # idx offset (units of 16 cols in G16)
                for g in range(NRANGES):
                    r0, r1 = g * RNG, min((g + 1) * RNG, NPAD)
                    src_ap = table[r0:r1, 0:32]
                    for ii, nch in enumerate(sched_g[g]):
                        nidx = nch * 128
                        gt = gt_p.tile([128, GATHER_CHUNKS, 32], dt.bfloat16,
                                       tag="gt")
                        _dma_gather_64b(
                            nc, gt[:, 0:nch, :], src_ap,
                            G16[:, ioff:ioff + nidx // 16], nidx, 32)
                        ioff += nidx // 16
                        for m0 in range(0, nch, 8):
                            mn = min(8, nch - m0)
                            oh = oh_p.tile([128, 8, NBUCK], dt.bfloat16, tag="oh")
                            j0 = ci + m0
                            nc.vector.tensor_tensor(
                                out=oh[:, 0:mn, :],
                                in0=DLF[:, j0:j0 + mn]
                                    .rearrange("p (c o) -> p c o", o=1)
                                    .to_broadcast([128, mn, NBUCK]),
                                in1=IO[:].rearrange("p (o s) -> p o s", o=1)
                                    .to_broadcast([128, mn, NBUCK]),
                                op=ALU.is_equal)
                            for s in range(mn):
                                b = chunk_meta[j0 + s][1]
                                nc.tensor.matmul(
                                    slot_ap(b), lhsT=oh[:, s, :],
                                    rhs=gt[:, m0 + s, 0:width],
                                    start=False, stop=True,
                                    skip_group_check=True)
                        ci += nch

            sched_g = sched

            # =========== LAYER 1 ===========
            # xs_pad is already the 256B-stride gather table (host-built)
            nc.vector.memset(T2S[:], 0.0)
            psum_zero([0, 1])
            aggregate(xs_t, 8, s1x_ap)

            # per-slab tail: u=dinv*S1x -> uT -> @W1 -> hs1 -> @W2 -> t2 slice
            for sl in range(NSLAB):
                par = sl & 1
                u = sl_p.tile([128, 8], dt.float32, tag="u")
                nc.vector.tensor_scalar_mul(u[:], s1x_slab(sl),
                                            DINV[:, sl:sl + 1])
                put = v128(2)[0:8, par, :]
                nc.tensor.transpose(out=put, in_=u[:], identity=IDN[:])
                uT = sl_p.tile([8, 128], dt.bfloat16, tag="uT")
                nc.scalar.copy(uT[:], put)
                ph = v32(4)[:, par, :]
                nc.tensor.matmul(ph, lhsT=uT[:], rhs=W1s[:],
                                 start=True, stop=True)
                # hs1 = relu(dinv*P + dinv*b1)
                hp = sl_p.tile([128, 32], dt.float32, tag="hp")
                nc.vector.scalar_tensor_tensor(
                    hp[:], ph, DINV[:, sl:sl + 1], DB1[:, sl, :],
                    op0=ALU.mult, op1=ALU.add)
                hs1 = sl_p.tile([128, 32], dt.float32, tag="hs1")
                nc.scalar.activation(hs1[:], hp[:], AF.Relu)
                # t2 = hs1 @ W2
                pt = v128(3)[0:32, par, :]
                nc.tensor.transpose(out=pt, in_=hs1[:], identity=IDN[:])
                hT = sl_p.tile([32, 128], dt.bfloat16, tag="hT")
                nc.scalar.copy(hT[:], pt)
                p2 = v32(4)[:, 2 + par, :]
                nc.tensor.matmul(p2, lhsT=hT[:], rhs=W2s[:],
                                 start=True, stop=True)
                nc.scalar.copy(T2S[:, sl, 0:32], p2)

            # slice (full 256B-stride rows) -> DRAM -> AllGather
            nc.sync.dma_start(
                agin[:, :].rearrange("(a p) b -> p a b", p=128), T2S[:])
            nc.gpsimd.collective_compute(
                "AllGather", ALU.bypass, replica_groups=[list(range(NCORES))],
                ins=[agin.opt()], outs=[agout.opt()])

            # =========== LAYER 2 ===========
            psum_zero(range(7))
            aggregate(agout, 32, s2_ap)

            # combine + max pool, batched per bank
            nc.vector.memset(GMAX[:], -1e30)
            for sl in range(NSLAB):
                h2p = sl_p.tile([128, 32], dt.float32, tag="h2p")
                nc.vector.scalar_tensor_tensor(
                    h2p[:], s2_slab(sl), DINV[:, sl:sl + 1], B2M[:, sl, :],
                    op0=ALU.mult, op1=ALU.add)
                h2 = sl_p.tile([128, 32], dt.float32, tag="h2")
                nc.scalar.activation(h2[:], h2p[:], AF.Relu)
                nc.vector.tensor_tensor(GMAX[:], GMAX[:], h2[:], op=ALU.max)

            # ---- head ----
            psg = SCR[:].rearrange("p (a f) -> p a f", f=128)[0:32, 0, :]
            nc.tensor.transpose(out=psg, in_=GMAX[:], identity=IDN[:])
            GT = per_p.tile([32, 128], dt.float32)
            nc.scalar.copy(GT[:], psg)
            GV = per_p.tile([32, 1], dt.float32)
            nc.vector.reduce_max(GV[:], GT[:], axis=AX.X)
            nc.sync.dma_start(arin[:, :], GV[:])
            nc.gpsimd.collective_compute(
                "AllReduce", ALU.max, replica_groups=[list(range(NCORES))],
                ins=[arin.opt()], outs=[arout.opt()])
            GAR = per_p.tile([32, 1], dt.float32)
            nc.sync.dma_start(GAR[:], arout[:, :])
            psl = SCR[:].rearrange("p (a f) -> p a f", f=32)[0:1, 15, 0:5]
            nc.tensor.matmul(psl, lhsT=GAR[:], rhs=FCW[:], start=True, stop=True)
            LG = per_p.tile([1, 5], dt.float32)
            nc.vector.tensor_add(LG[:], psl, FCB[:])
            MX = per_p.tile([1, 1], dt.float32)
            nc.vector.reduce_max(MX[:], LG[:], axis=AX.X)
            nc.vector.tensor_tensor(LG[:], LG[:], MX[:].to_broadcast([1, 5]),
                                    op=ALU.subtract)
            EX = per_p.tile([1, 5], dt.float32)
            nc.scalar.activation(EX[:], LG[:], AF.Exp)
            SM = per_p.tile([1, 1], dt.float32)
            nc.vector.reduce_sum(SM[:], EX[:], axis=AX.X)
            LS = per_p.tile([1, 1], dt.float32)
            nc.scalar.activation(LS[:], SM[:], AF.Ln)
            nc.vector.tensor_tensor(LG[:], LG[:], LS[:].to_broadcast([1, 5]),
                                    op=ALU.subtract)
            nc.sync.dma_start(out_t[:, :], LG[:])

    in_maps = []
    for k in range(NCORES):
        m = {"xs_pad": host["xs_pad"], "g16": percore[k]["g16"],
             "dlf": percore[k]["dlf"], "iota": host["iota"],
             "dinv_t": percore[k]["dinv_t"], "db1": percore[k]["db1"],
             "b2m": percore[k]["b2m"], "W1": host["W1"], "W2": host["W2"],
             "fcW": host["fcW"], "fcb": host["fcb"]}
        in_maps.append(m)
    return nc, in_maps


def kernel(x, edge_index, W1, b1, W2, b2, fcW, fcb):
    global LAST_RESULTS, RUN_WALL_NS
    nc, in_maps = build(x, edge_index, W1, b1, W2, b2, fcW, fcb)
    nc.compile()
    import os, time as _time
    trace = os.environ.get("GNN_TRACE", "0") == "1"
    _t0 = _time.time()
    res = run_bass_kernel_spmd(nc, in_maps, core_ids=list(range(NCORES)),
                               trace=trace)
    RUN_WALL_NS = int((_time.time() - _t0) * 1e9)
    LAST_RESULTS = res
    return res.results[0]["out"].astype(np.float32)
